# revision 47
# baseline (speedup 1.0000x reference)
"""Longformer encoder (12-layer, sliding-window attention) on 8 Trainium2 cores.

Sharding: (batch=4) x (seq half=2) -> 8 cores; 1024 tokens/core.
Sliding-window attention (+-256) exchanges edge K/V (post-projection) between
the two cores of each batch pair via a 2-rank AllGather (halo K/V are NOT
recomputed locally, unlike v1).

On-device layout is feature-major: activations [feature_partition, token].
Attention uses diagonal q=128 blocking: each 128-query tile attends exactly
5 key tiles of 128 (window +-256); only the 2 edge key tiles are banded and
their masks are accumulated into the scores PSUM via an identity matmul
(no vector mask adds at all).  All matmuls bf16, fp32 PSUM; LN/softmax fp32.
"""

import os
import numpy as np
import ml_dtypes

B, S, C = 4, 2048, 128
H, NH, HD, FF, W1 = 768, 12, 64, 3072, 256
L = int(os.environ.get("KERNEL_NL", "12"))
T = 1024            # tokens per core
HT = H // 128       # feature tiles (6)
FT = FF // 128      # ffn feature tiles (24)
VH = HD + 1         # v columns per head incl ones column (65)
VW = NH * VH        # v row width per token tile (780)
KW = T + 2 * W1     # padded key cols per feature tile (1536)
NKT = KW // 128     # padded key tiles (12)
EPS = 1e-5
NEG = -30000.0
ISQ = float(1.0 / np.sqrt(HD))

bf16 = ml_dtypes.bfloat16

_CACHED = {}
LAST_EXEC_NS = None


def _build(n_layers):
    import concourse.bacc as bacc
    import concourse.mybir as mybir
    from concourse import tile
    from contextlib import ExitStack

    dt = mybir.dt
    AF = mybir.ActivationFunctionType
    OP = mybir.AluOpType

    nc = bacc.Bacc(None, target_bir_lowering=False, debug=False)

    # ---------------- DRAM I/O ----------------
    xT = nc.dram_tensor("xT", [C, T], dt.bfloat16, kind="ExternalInput")
    pe = nc.dram_tensor("pe", [H, T], dt.float32, kind="ExternalInput")
    # 8 multiplicative 0/1 mask tiles [128 keys, 256 queries] for query-PAIR
    # blocking: 0:G0 1:G1 2:G4 3:G5 4:P0 5:P1 6:P2 7:P3
    # (G* generic pair left/right band edges; P* seq-edge specials)
    mks = nc.dram_tensor("mks", [8, 128, 256], dt.bfloat16, kind="ExternalInput")

    up_w1 = nc.dram_tensor("up_w1", [C, H], dt.bfloat16, kind="ExternalInput")
    up_w2 = nc.dram_tensor("up_w2", [H, H], dt.bfloat16, kind="ExternalInput")
    dn_w1 = nc.dram_tensor("dn_w1", [H, H], dt.bfloat16, kind="ExternalInput")
    dn_w2 = nc.dram_tensor("dn_w2", [H, C], dt.bfloat16, kind="ExternalInput")

    Wq = nc.dram_tensor("Wq", [n_layers, H, H], dt.bfloat16, kind="ExternalInput")
    Wk = nc.dram_tensor("Wk", [n_layers, H, H], dt.bfloat16, kind="ExternalInput")
    Wv = nc.dram_tensor("Wv", [n_layers, H, H], dt.bfloat16, kind="ExternalInput")
    Wo = nc.dram_tensor("Wo", [n_layers, H, H], dt.bfloat16, kind="ExternalInput")
    Wi = nc.dram_tensor("Wi", [n_layers, H, FF], dt.bfloat16, kind="ExternalInput")
    Wo2 = nc.dram_tensor("Wo2", [n_layers, FF, H], dt.bfloat16, kind="ExternalInput")

    # per-feature vectors, host-packed as [128, ntiles] (column j = feats 128j+)
    # order: bq bk (spare) boE g1 b1 bo2 g2 b2 pad
    vecs = nc.dram_tensor("vecs", [n_layers, 128, 10 * HT], dt.float32,
                          kind="ExternalInput")
    bi_all = nc.dram_tensor("bi_all", [n_layers, 128, FT], dt.float32,
                            kind="ExternalInput")
    # order: up_b1 up_b2 ln_g ln_b dn_b1 pad
    gvec = nc.dram_tensor("gvec", [128, 6 * HT], dt.float32, kind="ExternalInput")
    dn_b2 = nc.dram_tensor("dn_b2", [128, 1], dt.float32, kind="ExternalInput")

    y = nc.dram_tensor("y", [C, T], dt.float32, kind="ExternalOutput")

    # halo exchange: edge K (6 ft x 256 tok) and edge V (2 token tiles) per side
    KSEG = HT * 128 * 256          # 196608
    VSEG = 2 * 128 * VW            # 199680
    KL, KR, VL, VR = 0, KSEG, 2 * KSEG, 2 * KSEG + VSEG
    SEND_N = 2 * KSEG + 2 * VSEG
    cc_send = nc.dram_tensor("cc_send", [SEND_N], dt.bfloat16)
    cc_recv = nc.dram_tensor("cc_recv", [2, SEND_N], dt.bfloat16)

    with tile.TileContext(nc) as tc, ExitStack() as ctx:
        pp = ctx.enter_context(tc.tile_pool(name="persist", bufs=1))
        wp = ctx.enter_context(tc.tile_pool(name="w768", bufs=12))
        wip = ctx.enter_context(tc.tile_pool(name="wi512", bufs=8))
        bp = ctx.enter_context(tc.tile_pool(name="bias", bufs=2))
        fp = ctx.enter_context(tc.tile_pool(name="ffmid", bufs=6))
        rp = ctx.enter_context(tc.tile_pool(name="rows", bufs=2))
        pbp = ctx.enter_context(tc.tile_pool(name="pbuf", bufs=2))
        psW = ctx.enter_context(tc.tile_pool(name="psW", bufs=2, space="PSUM"))

        # ---------------- persistent tiles ----------------
        h = pp.tile([128, HT * T], dt.float32, tag="h")          # residual stream
        hb = pp.tile([128, HT * T], dt.bfloat16, tag="hb")       # bf16 stream
        qb = pp.tile([128, HT * T], dt.bfloat16, tag="qb")       # Q feature-major
        Kp = pp.tile([128, HT * KW], dt.bfloat16, tag="Kp")      # K padded
        Vp = pp.tile([128, NKT * VW], dt.bfloat16, tag="Vp")     # V padded tok-major
        ob = pp.tile([128, HT * T], dt.bfloat16, tag="ob")       # attn out / scratch
        mk = pp.tile([128, 8 * 256], dt.bfloat16, tag="mk")      # 0/1 masks
        ones_c = pp.tile([128, 1], dt.bfloat16, tag="ones_c")
        oh_c = pp.tile([128, 1], dt.bfloat16, tag="oh_c")        # 1/H column
        ones_r = pp.tile([1, 128], dt.bfloat16, tag="ones_r")
        gv = pp.tile([128, 6 * HT], dt.float32, tag="gv")
        dnb2 = pp.tile([128, 1], dt.float32, tag="dnb2")

        def hs(ft, qt=None):
            if qt is None:
                return slice(ft * T, (ft + 1) * T)
            return slice(ft * T + qt * 512, ft * T + qt * 512 + 512)

        def k_ap(ft, kt):   # [128,128] padded key tile kt (0..11) of feature ft
            return Kp[:, ft * KW + kt * 128: ft * KW + (kt + 1) * 128]

        def v_ap(kt):       # [128, VW] padded V token tile kt
            return Vp[:, kt * VW:(kt + 1) * VW]



        nc.vector.memset(ones_c[:], 1.0)
        nc.vector.memset(ones_r[:], 1.0)
        nc.vector.memset(oh_c[:], 1.0 / H)
        for kt in range(NKT):
            vre = v_ap(kt).rearrange("p (n c) -> p n c", c=VH)
            nc.vector.memset(vre[:, :, HD:HD + 1], 1.0)

        nc.sync.dma_start(out=gv[:], in_=gvec[:])
        nc.sync.dma_start(out=dnb2[:], in_=dn_b2[:])
        for i in range(8):
            nc.sync.dma_start(out=mk[:, i * 256:(i + 1) * 256], in_=mks[i])

        # ---------------- helpers ----------------
        def w_proj(w_dram, rhs_cols, out_fn, qts=(0, 1), wtiles=None):
            """out_fn(mt, qt, psum) for psum = W.T @ rhs over HT k-tiles."""
            if wtiles is None:
                wtiles = []
                for k in range(HT):
                    t = wp.tile([128, H], dt.bfloat16, tag="w768")
                    nc.sync.dma_start(out=t[:], in_=w_dram[k * 128:(k + 1) * 128, :])
                    wtiles.append(t)
            for qt in qts:
                for mt in range(HT):
                    pt = psW.tile([128, 512], dt.float32, tag="work")
                    for k in range(HT):
                        nc.tensor.matmul(
                            pt[:], wtiles[k][:, mt * 128:(mt + 1) * 128],
                            rhs_cols(k, qt),
                            start=(k == 0), stop=(k == HT - 1))
                    out_fn(mt, qt, pt)
            return wtiles

        # layernorm of one 512-token half qt: updates h, writes hb.
        # hb slices hold the pre-norm bf16 copy during stats (overwritten with
        # the normalized value at the end); ob slices are square scratch.
        # rsqrt is computed on DVE (magic-constant seed + Newton) so the
        # scalar engine never loads the Ln/Sqrt activation tables.
        MAGIC = float(0x5F3759DF)

        def layernorm_half(g_col, b_col, qt):
            # stats inputs: bf16 copy of h + its squares. columns pre-scaled
            # by 1/H so mp/sp are E[x] / E[x^2] directly.
            for ft in range(HT):
                nc.vector.tensor_copy(hb[:, hs(ft, qt)], h[:, hs(ft, qt)])
            for ft in range(HT):
                nc.vector.tensor_mul(ob[:, hs(ft, qt)], hb[:, hs(ft, qt)],
                                     hb[:, hs(ft, qt)])
            mp = psW.tile([1, 512], dt.float32, tag="work", name="mp")
            for ft in range(HT):
                nc.tensor.matmul(mp[:], oh_c[:], hb[:, hs(ft, qt)],
                                 start=(ft == 0), stop=(ft == HT - 1))
            sp = psW.tile([1, 512], dt.float32, tag="work", name="sp")
            for ft in range(HT):
                nc.tensor.matmul(sp[:], oh_c[:], ob[:, hs(ft, qt)],
                                 start=(ft == 0), stop=(ft == HT - 1))
            mrow = rp.tile([1, 512], dt.bfloat16, tag="mrow")
            nc.vector.tensor_copy(mrow[:], mp[:])
            m2 = rp.tile([1, 512], dt.float32, tag="m2")
            nc.vector.tensor_mul(m2[:], mp[:], mrow[:])
            spn = rp.tile([1, 512], dt.float32, tag="spn")
            nc.vector.tensor_scalar_add(spn[:], sp[:], EPS)
            var = rp.tile([1, 512], dt.float32, tag="var")
            nc.vector.scalar_tensor_tensor(var[:], m2[:], -1.0, spn[:],
                                           OP.mult, OP.add)
            rrow = rp.tile([1, 512], dt.bfloat16, tag="rrow")
            # y0 = bitcast(MAGIC - (bits(var) >> 1)); one Newton step.
            sd = rp.tile([1, 512], dt.float32, tag="sd")
            nc.vector.tensor_scalar(sd[:].bitcast(dt.int32),
                                    var[:].bitcast(dt.int32), -0.5,
                                    MAGIC, OP.mult, OP.add)
            y0 = sd[:]
            t1 = rp.tile([1, 512], dt.float32, tag="t1")
            nc.vector.tensor_mul(t1[:], y0, y0)
            nc.vector.tensor_mul(t1[:], t1[:], var[:])
            nc.vector.tensor_scalar(t1[:], t1[:], -0.5, 1.5, OP.mult, OP.add)
            nc.vector.tensor_mul(rrow[:], y0, t1[:])
            mb = psW.tile([128, 512], dt.float32, tag="work", name="mb")
            nc.tensor.matmul(mb[:], ones_r[:], mrow[:], start=True, stop=True)
            rb = psW.tile([128, 512], dt.float32, tag="work", name="rb")
            nc.tensor.matmul(rb[:], ones_r[:], rrow[:], start=True, stop=True)
            mbs = rp.tile([128, 512], dt.bfloat16, tag="mbs")
            nc.scalar.activation(mbs[:], mb[:], AF.Copy)
            rbs = rp.tile([128, 512], dt.bfloat16, tag="rbs")
            nc.scalar.activation(rbs[:], rb[:], AF.Copy)
            for ft in range(HT):
                sl = hs(ft, qt)
                t2 = rp.tile([128, 512], dt.float32, tag="t2")
                nc.vector.tensor_sub(t2[:], h[:, sl], mbs[:])
                nc.vector.tensor_mul(t2[:], t2[:], rbs[:])
                nc.scalar.activation(h[:, sl], t2[:], AF.Identity,
                                     scale=g_col(ft), bias=b_col(ft))
                nc.scalar.activation(hb[:, sl], t2[:], AF.Identity,
                                     scale=g_col(ft), bias=b_col(ft))

        # ---------------- input projection ----------------
        for ft in range(HT):
            nc.sync.dma_start(out=h[:, hs(ft)], in_=pe[ft * 128:(ft + 1) * 128, :])
        xb = rp.tile([128, T], dt.bfloat16, tag="xb", bufs=1)
        nc.sync.dma_start(out=xb[:], in_=xT[:])

        w1t = wp.tile([128, H], dt.bfloat16, tag="w768")
        nc.sync.dma_start(out=w1t[:], in_=up_w1[:])
        for mt in range(HT):
            for qt in range(2):
                pt = psW.tile([128, 512], dt.float32, tag="work")
                nc.tensor.matmul(pt[:], w1t[:, mt * 128:(mt + 1) * 128],
                                 xb[:, qt * 512:qt * 512 + 512],
                                 start=True, stop=True)
                nc.scalar.activation(ob[:, hs(mt, qt)], pt[:], AF.Tanh,
                                     bias=gv[:, 0 * HT + mt:0 * HT + mt + 1])

        def up2_out(mt, qt, pt):
            sl = hs(mt, qt)
            nc.vector.scalar_tensor_tensor(
                h[:, sl], pt[:], gv[:, 1 * HT + mt:1 * HT + mt + 1], h[:, sl],
                OP.add, OP.add)
        w_proj(up_w2, lambda k, qt: ob[:, hs(k, qt)], up2_out)

        for qt in range(2):
            layernorm_half(lambda ft: gv[:, 2 * HT + ft:2 * HT + ft + 1],
                           lambda ft: gv[:, 3 * HT + ft:3 * HT + ft + 1], qt)

        # ---------------- encoder layers ----------------
        for l in range(n_layers):
            bv_t = bp.tile([128, 10 * HT], dt.float32, tag="bvec")
            nc.sync.dma_start(out=bv_t[:], in_=vecs[l])
            bi_t = bp.tile([128, FT], dt.float32, tag="bivec")
            nc.sync.dma_start(out=bi_t[:], in_=bi_all[l])

            def vcol(i, ft):
                return bv_t[:, i * HT + ft:i * HT + ft + 1]

            # --- K projection (interior only; evict into padded interior) ---
            def k_out(mt, qt, pt):
                sl = slice(mt * KW + 256 + qt * 512, mt * KW + 256 + qt * 512 + 512)
                nc.vector.tensor_scalar_add(Kp[:, sl], pt[:], vcol(1, mt))
            w_proj(Wk[l], lambda k, qt: hb[:, hs(k, qt)], k_out)

            # --- Q projection ---
            def q_out(mt, qt, pt):
                nc.vector.tensor_scalar_add(qb[:, hs(mt, qt)], pt[:], vcol(0, mt))
            w_proj(Wq[l], lambda k, qt: hb[:, hs(k, qt)], q_out)

            # --- V projection (interior, token-major: h stationary) ---
            wvt = []
            for k in range(HT):
                t = wp.tile([128, H], dt.bfloat16, tag="w768")
                nc.sync.dma_start(out=t[:], in_=Wv[l, k * 128:(k + 1) * 128, :])
                wvt.append(t)
            for vt in range(2, 10):
                tt = vt - 2
                for n0, nn in ((0, 512), (512, 256)):
                    pt = psW.tile([128, 512], dt.float32, tag="work")
                    for k in range(HT):
                        nc.tensor.matmul(
                            pt[:, :nn],
                            hb[:, k * T + tt * 128: k * T + tt * 128 + 128],
                            wvt[k][:, n0:n0 + nn],
                            start=(k == 0), stop=(k == HT - 1))
                    dst = v_ap(vt).rearrange("p (n c) -> p n c", c=VH)
                    h0, nh_ = n0 // HD, nn // HD
                    src = pt[:, :nn].rearrange("p (n c) -> p n c", c=HD)
                    nc.scalar.activation(dst[:, h0:h0 + nh_, 0:HD], src[:], AF.Copy)

            # --- attention, grouped by 512-token halves ---
            att_cm = tc.tile_pool(name="attps", bufs=2, space="PSUM")
            att_ps = att_cm.__enter__()

            # one query PAIR (q tiles 2p, 2p+1; 256 queries) vs its 6 key
            # tiles, N=256 matmuls. The 0/1 masks zero out-of-window slots
            # (incl. each query's missing edge kt), which also makes the
            # uniform start/stop PV accumulation over all 6 kt correct.
            def attn_pair(p, hd_i, pv, qoff):
                ft, r0 = hd_i // 2, (hd_i % 2) * HD
                q0 = ft * T + 2 * p * 128
                st = att_ps.tile([128, 1536], dt.float32, tag="st", bufs=1)
                for j in range(6):
                    nc.tensor.matmul(
                        st[:, j * 256:(j + 1) * 256],
                        k_ap(ft, 2 * p + j)[r0:r0 + HD, :],
                        qb[r0:r0 + HD, q0:q0 + 256],
                        start=True, stop=True)
                pe_ = pbp.tile([128, 1536], dt.bfloat16, tag="pb")
                nc.scalar.activation(pe_[:], st[:], AF.Exp, scale=ISQ)
                lb = 4 if p == 0 else 0
                rb = 6 if p == 3 else 2
                nc.vector.tensor_mul(pe_[:, 0:512], pe_[:, 0:512],
                                     mk[:, lb * 256:(lb + 2) * 256])
                nc.vector.tensor_mul(pe_[:, 1024:1536], pe_[:, 1024:1536],
                                     mk[:, rb * 256:(rb + 2) * 256])
                for j in range(6):
                    nc.tensor.matmul(
                        pv[:, qoff:qoff + 256],
                        v_ap(2 * p + j)[:, hd_i * VH:(hd_i + 1) * VH],
                        pe_[:, j * 256:(j + 1) * 256],
                        start=(j == 0), stop=(j == 5))

            def attn_norm(segs, w, hd_i, pv):
                ft, r0 = hd_i // 2, (hd_i % 2) * HD
                dn = rp.tile([1, 512], dt.float32, tag="dnr")
                nc.scalar.activation(dn[0:1, 0:w], pv[HD:VH, 0:w], AF.Copy)
                r = rp.tile([1, 512], dt.float32, tag="rrec")
                nc.vector.reciprocal_approx_fast(r[0:1, 0:w], dn[0:1, 0:w])
                rb_ = rp.tile([1, 512], dt.bfloat16, tag="rbb")
                nc.vector.tensor_copy(rb_[0:1, 0:w], r[0:1, 0:w])
                bc = psW.tile([HD, 512], dt.float32, tag="work", name="bc")
                nc.tensor.matmul(bc[:, 0:w], ones_r[:, 0:HD], rb_[0:1, 0:w],
                                 start=True, stop=True)
                bcs = rp.tile([HD, 512], dt.bfloat16, tag="bcs")
                nc.scalar.activation(bcs[:, 0:w], bc[:, 0:w], AF.Copy)
                for pc, oc, sw in segs:
                    nc.vector.tensor_mul(
                        ob[r0:r0 + HD, ft * T + oc: ft * T + oc + sw],
                        pv[0:HD, pc:pc + sw], bcs[:, pc:pc + sw])

            def o_out_g(g):
                def o_out(mt, qt, pt):
                    sl = hs(mt, qt)
                    nc.vector.scalar_tensor_tensor(
                        h[:, sl], pt[:], vcol(3, mt), h[:, sl], OP.add, OP.add)
                return o_out

            # FFN split: ffn1 produces fmid chunks (kept in fp pool),
            # ffn2 consumes them into the fpt accumulators.
            def ffn1_half(g, store):
                for ch in range(6):
                    wi_ch = []
                    for k in range(HT):
                        t = wip.tile([128, 512], dt.bfloat16, tag="wi")
                        nc.sync.dma_start(
                            out=t[:],
                            in_=Wi[l, k * 128:(k + 1) * 128,
                                   ch * 512:(ch + 1) * 512])
                        wi_ch.append(t)
                    fmid = fp.tile([128, 4 * 512], dt.bfloat16, tag="fmid")
                    store.append(fmid)
                    for mi in range(4):
                        mt = ch * 4 + mi
                        pt = psW.tile([128, 512], dt.float32, tag="work")
                        for k in range(HT):
                            nc.tensor.matmul(
                                pt[:], wi_ch[k][:, mi * 128:(mi + 1) * 128],
                                hb[:, hs(k, g)],
                                start=(k == 0), stop=(k == HT - 1))
                        nc.scalar.activation(fmid[:, mi * 512:(mi + 1) * 512],
                                             pt[:], AF.Gelu,
                                             bias=bi_t[:, mt:mt + 1])

            def ffn2_half(g, store, ffn_ps):
                fpt = [ffn_ps.tile([128, 512], dt.float32, tag=f"ff2_{m}",
                                   name=f"ff2_{m}_{g}") for m in range(HT)]
                for ch in range(6):
                    fmid = store[ch]
                    wo2_ch = []
                    for mi in range(4):
                        t = wp.tile([128, H], dt.bfloat16, tag="w768")
                        nc.sync.dma_start(
                            out=t[:],
                            in_=Wo2[l, (ch * 4 + mi) * 128:(ch * 4 + mi + 1) * 128, :])
                        wo2_ch.append(t)
                    for m in range(HT):
                        for mi in range(4):
                            kt = ch * 4 + mi
                            nc.tensor.matmul(
                                fpt[m][:], wo2_ch[mi][:, m * 128:(m + 1) * 128],
                                fmid[:, mi * 512:(mi + 1) * 512],
                                start=(kt == 0), stop=(kt == FT - 1))
                for m in range(HT):
                    sl = hs(m, g)
                    nc.vector.scalar_tensor_tensor(h[:, sl], fpt[m][:], vcol(6, m),
                                                   h[:, sl], OP.add, OP.add)

            # ---- emission schedule ----
            # attention g0 (interior qtiles first), then O/LN1/FFN1 g0
            # overlapping attention g1.
            fm0, fm1 = [], []
            # interior quad (qi 2..5, tokens 256:768): no halo dependency ->
            # overlaps the AllGather emitted after it.
            for hd_i in range(NH):
                pv = att_ps.tile([VH, 512], dt.float32, tag="pv",
                                 name=f"pvI_{hd_i}", bufs=2)
                for p in (1, 2):
                    attn_pair(p, hd_i, pv, (p - 1) * 256)
                attn_norm([(0, 256, 512)], 512, hd_i, pv)
            # --- K/V edge exchange (K edges: tok 0:256 and 768:1024) ---
            for ft in range(HT):
                nc.sync.dma_start(
                    out=cc_send[KL + ft * 128 * 256: KL + (ft + 1) * 128 * 256]
                    .rearrange("(p t) -> p t", p=128),
                    in_=Kp[:, ft * KW + 256: ft * KW + 512])
                nc.sync.dma_start(
                    out=cc_send[KR + ft * 128 * 256: KR + (ft + 1) * 128 * 256]
                    .rearrange("(p t) -> p t", p=128),
                    in_=Kp[:, ft * KW + 1024: ft * KW + 1280])
            nc.sync.dma_start(
                out=cc_send[VL:VL + VSEG].rearrange("(p t) -> p t", p=128),
                in_=Vp[:, 2 * VW:4 * VW])
            nc.sync.dma_start(
                out=cc_send[VR:VR + VSEG].rearrange("(p t) -> p t", p=128),
                in_=Vp[:, 8 * VW:10 * VW])
            nc.gpsimd.collective_compute(
                "AllGather", OP.bypass, ins=[cc_send[:]], outs=[cc_recv[:]],
                replica_groups=[[0, 1], [2, 3], [4, 5], [6, 7]])
            # left halo <- rank0's right edge ; right halo <- rank1's left edge
            for ft in range(HT):
                nc.sync.dma_start(
                    out=Kp[:, ft * KW: ft * KW + 256],
                    in_=cc_recv[0, KR + ft * 128 * 256: KR + (ft + 1) * 128 * 256]
                    .rearrange("(p t) -> p t", p=128))
                nc.sync.dma_start(
                    out=Kp[:, ft * KW + 1280: ft * KW + 1536],
                    in_=cc_recv[1, KL + ft * 128 * 256: KL + (ft + 1) * 128 * 256]
                    .rearrange("(p t) -> p t", p=128))
            nc.sync.dma_start(
                out=Vp[:, 0:2 * VW],
                in_=cc_recv[0, VR:VR + VSEG].rearrange("(p t) -> p t", p=128))
            nc.sync.dma_start(
                out=Vp[:, 10 * VW:12 * VW],
                in_=cc_recv[1, VL:VL + VSEG].rearrange("(p t) -> p t", p=128))

            # edge pairs (p=0 left halo / p=3 right halo) after the CC.
            for hd_i in range(NH):
                pv = att_ps.tile([VH, 512], dt.float32, tag="pv",
                                 name=f"pvE_{hd_i}", bufs=2)
                for p in (3, 0):
                    attn_pair(p, hd_i, pv, 0 if p == 0 else 256)
                attn_norm([(0, 0, 256), (256, 768, 256)], 512, hd_i, pv)
            wo_t = w_proj(Wo[l], lambda k, qt: ob[:, hs(k, qt)],
                          o_out_g(0), qts=(0,))
            layernorm_half(lambda ft: vcol(4, ft), lambda ft: vcol(5, ft), 0)
            # FFN1(g0) emitted below attention-g1 in priority: its dense
            # matmuls fill the exp-latency bubbles of attention g1.
            ffn1_half(0, fm0)
            w_proj(Wo[l], lambda k, qt: ob[:, hs(k, qt)], o_out_g(1), qts=(1,),
                   wtiles=wo_t)
            layernorm_half(lambda ft: vcol(4, ft), lambda ft: vcol(5, ft), 1)
            att_cm.__exit__(None, None, None)

            ffn_cm = tc.tile_pool(name="ffps", bufs=1, space="PSUM")
            ffn_ps = ffn_cm.__enter__()
            ffn2_half(0, fm0, ffn_ps)
            ffn1_half(1, fm1)
            layernorm_half(lambda ft: vcol(7, ft), lambda ft: vcol(8, ft), 0)
            ffn2_half(1, fm1, ffn_ps)
            ffn_cm.__exit__(None, None, None)
            layernorm_half(lambda ft: vcol(7, ft), lambda ft: vcol(8, ft), 1)

        # ---------------- output projection ----------------
        def d1_out(mt, qt, pt):
            nc.scalar.activation(ob[:, hs(mt, qt)], pt[:], AF.Tanh,
                                 bias=gv[:, 4 * HT + mt:4 * HT + mt + 1])
        w_proj(dn_w1, lambda k, qt: hb[:, hs(k, qt)], d1_out)

        w2t = wp.tile([128, HT * C], dt.bfloat16, tag="w768")
        for k in range(HT):
            nc.sync.dma_start(out=w2t[:, k * C:(k + 1) * C],
                              in_=dn_w2[k * 128:(k + 1) * 128, :])
        for qt in range(2):
            pt = psW.tile([128, 512], dt.float32, tag="work")
            for k in range(HT):
                nc.tensor.matmul(pt[:], w2t[:, k * C:(k + 1) * C],
                                 ob[:, hs(k, qt)], start=(k == 0),
                                 stop=(k == HT - 1))
            yo = rp.tile([128, 512], dt.float32, tag="yout")
            nc.scalar.activation(yo[:], pt[:], AF.Identity, bias=dnb2[:])
            nc.sync.dma_start(out=y[:, qt * 512:qt * 512 + 512], in_=yo[:])

    nc.compile()
    return nc


def _host_prep(inputs, n_layers):
    f32 = np.float32
    x = np.asarray(inputs["x"], f32)
    ts = np.asarray(inputs["timesteps"])
    half = C // 2
    freqs = np.exp(-np.log(10000.0) * np.arange(half, dtype=f32) / half)
    a = ts.astype(f32)[:, None] * freqs[None, :]
    emb0 = np.concatenate([np.cos(a), np.sin(a)], -1).astype(f32)
    t1 = emb0 @ np.asarray(inputs["t_w1"], f32) + np.asarray(inputs["t_b1"], f32)
    t1 = t1 / (1.0 + np.exp(-t1))
    emb = (t1 @ np.asarray(inputs["t_w2"], f32) + np.asarray(inputs["t_b2"], f32)).astype(f32)

    def cvt(w):
        return np.ascontiguousarray(np.asarray(w, f32).astype(bf16))

    def packvec(v, nt):
        return np.ascontiguousarray(np.asarray(v, f32).reshape(nt, 128).T)

    com = dict(
        up_w1=cvt(inputs["up_w1"]), up_w2=cvt(inputs["up_w2"]),
        dn_w1=cvt(inputs["down_w1"]), dn_w2=cvt(inputs["down_w2"]),
        Wq=cvt(inputs["Wq"][:n_layers]), Wk=cvt(inputs["Wk"][:n_layers]),
        Wv=cvt(inputs["Wv"][:n_layers]), Wo=cvt(inputs["Wo"][:n_layers]),
        Wi=cvt(inputs["Wi"][:n_layers]), Wo2=cvt(inputs["Wo2"][:n_layers]),
        dn_b2=np.ascontiguousarray(np.asarray(inputs["down_b2"], f32).reshape(1, C).T),
    )
    # bv folded into bo: boE = bo + bv @ Wo
    boE = np.stack([
        np.asarray(inputs["bo"], f32)[ll]
        + np.asarray(inputs["bv"], f32)[ll] @ np.asarray(inputs["Wo"], f32)[ll]
        for ll in range(n_layers)])
    zero = np.zeros((n_layers, H), f32)
    vec_srcs = [np.asarray(inputs["bq"], f32)[:n_layers],
                np.asarray(inputs["bk"], f32)[:n_layers], zero, boE,
                np.asarray(inputs["g1"], f32)[:n_layers],
                np.asarray(inputs["b1"], f32)[:n_layers],
                np.asarray(inputs["bo2"], f32)[:n_layers],
                np.asarray(inputs["g2"], f32)[:n_layers],
                np.asarray(inputs["b2"], f32)[:n_layers], zero]
    vecs = np.stack([
        np.concatenate([packvec(src[ll], HT) for src in vec_srcs], axis=1)
        for ll in range(n_layers)])
    com["vecs"] = np.ascontiguousarray(vecs.astype(f32))
    com["bi_all"] = np.ascontiguousarray(
        np.stack([packvec(np.asarray(inputs["bi"], f32)[ll], FT)
                  for ll in range(n_layers)]).astype(f32))
    com["gvec"] = np.ascontiguousarray(np.concatenate([
        packvec(inputs["up_b1"], HT), packvec(inputs["up_b2"], HT),
        packvec(inputs["ln_g"], HT), packvec(inputs["ln_b"], HT),
        packvec(inputs["down_b1"], HT), packvec(inputs["down_b1"], HT)],
        axis=1).astype(f32))

    pos = np.asarray(inputs["pos_emb"], f32)

    # multiplicative 0/1 mask tiles [8,128,256] for query-pair blocking:
    # rows = key-in-tile, cols = [query tile 2p | query tile 2p+1].
    pi = np.arange(128)[:, None]
    fi = np.arange(128)[None, :]
    ML = (pi >= fi).astype(f32)
    MR = (pi <= fi).astype(f32)
    ZZ = np.ones((128, 128), f32)
    NN = np.zeros((128, 128), f32)

    def cat(a, b):
        return np.concatenate([a, b], axis=1)

    G0, G1 = cat(ML, NN), cat(ZZ, ML)
    G4, G5 = cat(MR, ZZ), cat(NN, MR)
    mk_sh = {}
    # per-shard specials: P0/P1 (left seq edge, pair 0), P2/P3 (right, pair 3)
    # sh0: q0kt0=NN q0kt1=NN q1kt1=NN ; q6kt10=MR q7kt10=ZZ q7kt11=MR
    mk_sh[0] = np.stack([G0, G1, G4, G5,
                         cat(NN, NN), cat(NN, NN),
                         cat(MR, ZZ), cat(NN, MR)])
    # sh1: q0kt0=ML q0kt1=ZZ q1kt1=ML ; right specials all zero
    mk_sh[1] = np.stack([G0, G1, G4, G5,
                         cat(ML, NN), cat(ZZ, ML),
                         cat(NN, NN), cat(NN, NN)])
    for sh in mk_sh:
        mk_sh[sh] = np.ascontiguousarray(mk_sh[sh].astype(bf16))

    in_maps = []
    for c in range(8):
        b, sh = c // 2, c % 2
        sl = slice(sh * T, (sh + 1) * T)
        im = dict(com)
        im["xT"] = np.ascontiguousarray(x[b, sl].T.astype(bf16))
        im["pe"] = np.ascontiguousarray((pos[sl] + emb[b][None, :]).T.astype(f32))
        im["mks"] = mk_sh[sh]
        in_maps.append(im)
    return in_maps


def kernel(**inputs):
    from concourse.bass_utils import run_bass_kernel_spmd

    n_layers = L
    if n_layers not in _CACHED:
        _CACHED[n_layers] = _build(n_layers)
    nc = _CACHED[n_layers]
    in_maps = _host_prep(inputs, n_layers)
    trace = os.environ.get("KERNEL_TRACE", "0") == "1"
    tmpdir = os.environ.get("KERNEL_TMPDIR") or None
    res = run_bass_kernel_spmd(nc, in_maps, list(range(8)), trace=trace,
                               tmpdir=tmpdir)
    global LAST_EXEC_NS
    if getattr(res, "exec_time_ns", None):
        LAST_EXEC_NS = res.exec_time_ns
    out = np.empty((B, S, C), np.float32)
    for c in range(8):
        b, sh = c // 2, c % 2
        out[b, sh * T:(sh + 1) * T, :] = res.results[c]["y"].T
    return out



# revision 58
# speedup vs baseline: 1.2499x; 1.2499x over previous
"""Longformer encoder (12-layer, sliding-window attention) on 8 Trainium2 cores.

Sharding: (batch=4) x (seq half=2) -> 8 cores; 1024 tokens/core.
Sliding-window attention (+-256) exchanges edge K/V (post-projection) between
the two cores of each batch pair via a 2-rank AllGather (halo K/V are NOT
recomputed locally, unlike v1).

On-device layout is feature-major: activations [feature_partition, token].
Attention uses diagonal q=128 blocking: each 128-query tile attends exactly
5 key tiles of 128 (window +-256); only the 2 edge key tiles are banded and
their masks are accumulated into the scores PSUM via an identity matmul
(no vector mask adds at all).  All matmuls bf16, fp32 PSUM; LN/softmax fp32.
"""

import os
import numpy as np
import ml_dtypes

B, S, C = 4, 2048, 128
H, NH, HD, FF, W1 = 768, 12, 64, 3072, 256
L = int(os.environ.get("KERNEL_NL", "12"))
T = 1024            # tokens per core
HT = H // 128       # feature tiles (6)
FT = FF // 128      # ffn feature tiles (24)
VH = HD + 1         # v columns per head incl ones column (65)
VW = NH * VH        # v row width per token tile (780)
KW = T + 2 * W1     # padded key cols per feature tile (1536)
NKT = KW // 128     # padded key tiles (12)
EPS = 1e-5
NEG = -30000.0
ISQ = float(1.0 / np.sqrt(HD))

bf16 = ml_dtypes.bfloat16

_CACHED = {}
LAST_EXEC_NS = None


def _build(n_layers):
    import concourse.bacc as bacc
    import concourse.mybir as mybir
    from concourse import tile
    from contextlib import ExitStack

    dt = mybir.dt
    AF = mybir.ActivationFunctionType
    OP = mybir.AluOpType

    nc = bacc.Bacc(None, target_bir_lowering=False, debug=False)

    # ---------------- DRAM I/O ----------------
    xT = nc.dram_tensor("xT", [C, T], dt.bfloat16, kind="ExternalInput")
    pe = nc.dram_tensor("pe", [H, T], dt.float32, kind="ExternalInput")
    # 8 multiplicative 0/1 mask tiles: 0:ML 1:MR 2:q0kt0 3:q0kt1 4:q1kt1
    # 5:q7kt11 6:q7kt10 7:q6kt10
    mks = nc.dram_tensor("mks", [8, 128, 128], dt.bfloat16, kind="ExternalInput")

    up_w1 = nc.dram_tensor("up_w1", [C, H], dt.bfloat16, kind="ExternalInput")
    up_w2 = nc.dram_tensor("up_w2", [H, H], dt.bfloat16, kind="ExternalInput")
    dn_w1 = nc.dram_tensor("dn_w1", [H, H], dt.bfloat16, kind="ExternalInput")
    dn_w2 = nc.dram_tensor("dn_w2", [H, C], dt.bfloat16, kind="ExternalInput")

    Wq = nc.dram_tensor("Wq", [n_layers, H, H], dt.bfloat16, kind="ExternalInput")
    Wk = nc.dram_tensor("Wk", [n_layers, H, H], dt.bfloat16, kind="ExternalInput")
    Wv = nc.dram_tensor("Wv", [n_layers, H, H], dt.bfloat16, kind="ExternalInput")
    Wo = nc.dram_tensor("Wo", [n_layers, H, H], dt.bfloat16, kind="ExternalInput")
    Wi = nc.dram_tensor("Wi", [n_layers, H, FF], dt.bfloat16, kind="ExternalInput")
    Wo2 = nc.dram_tensor("Wo2", [n_layers, FF, H], dt.bfloat16, kind="ExternalInput")

    # per-feature vectors, host-packed as [128, ntiles] (column j = feats 128j+)
    # order: bq bk (spare) boE g1 b1 bo2 g2 b2 pad
    vecs = nc.dram_tensor("vecs", [n_layers, 128, 10 * HT], dt.float32,
                          kind="ExternalInput")
    bi_all = nc.dram_tensor("bi_all", [n_layers, 128, FT], dt.float32,
                            kind="ExternalInput")
    # order: up_b1 up_b2 ln_g ln_b dn_b1 pad
    gvec = nc.dram_tensor("gvec", [128, 6 * HT], dt.float32, kind="ExternalInput")
    dn_b2 = nc.dram_tensor("dn_b2", [128, 1], dt.float32, kind="ExternalInput")

    y = nc.dram_tensor("y", [C, T], dt.float32, kind="ExternalOutput")

    # halo exchange: edge K (6 ft x 256 tok) and edge V (2 token tiles) per side
    KSEG = HT * 128 * 256          # 196608
    VSEG = 2 * 128 * VW            # 199680
    KL, KR, VL, VR = 0, KSEG, 2 * KSEG, 2 * KSEG + VSEG
    SEND_N = 2 * KSEG + 2 * VSEG
    cc_send = nc.dram_tensor("cc_send", [SEND_N], dt.bfloat16)
    cc_recv = nc.dram_tensor("cc_recv", [2, SEND_N], dt.bfloat16)

    with tile.TileContext(nc) as tc, ExitStack() as ctx:
        pp = ctx.enter_context(tc.tile_pool(name="persist", bufs=1))
        wp = ctx.enter_context(tc.tile_pool(name="w768", bufs=14))
        wip = ctx.enter_context(tc.tile_pool(name="wi512", bufs=8))
        bp = ctx.enter_context(tc.tile_pool(name="bias", bufs=2))
        fp = ctx.enter_context(tc.tile_pool(name="ffmid", bufs=6))
        rp = ctx.enter_context(tc.tile_pool(name="rows", bufs=2))
        pbp = ctx.enter_context(tc.tile_pool(name="pbuf", bufs=3))
        psW = ctx.enter_context(tc.tile_pool(name="psW", bufs=2, space="PSUM"))

        # ---------------- persistent tiles ----------------
        h = pp.tile([128, HT * T], dt.float32, tag="h")          # residual stream
        hb = pp.tile([128, HT * T], dt.bfloat16, tag="hb")       # bf16 stream
        qb = pp.tile([128, HT * T], dt.bfloat16, tag="qb")       # Q feature-major
        Kp = pp.tile([128, HT * KW], dt.bfloat16, tag="Kp")      # K padded
        Vp = pp.tile([128, NKT * VW], dt.bfloat16, tag="Vp")     # V padded tok-major
        ob = pp.tile([128, HT * T], dt.bfloat16, tag="ob")       # attn out / scratch
        mk = pp.tile([128, 8 * 128], dt.bfloat16, tag="mk")      # 0/1 masks
        ones_c = pp.tile([128, 1], dt.bfloat16, tag="ones_c")
        oh_c = pp.tile([128, 1], dt.bfloat16, tag="oh_c")        # 1/H column
        ones_r = pp.tile([1, 128], dt.bfloat16, tag="ones_r")
        gv = pp.tile([128, 6 * HT], dt.float32, tag="gv")
        dnb2 = pp.tile([128, 1], dt.float32, tag="dnb2")

        def hs(ft, qt=None):
            if qt is None:
                return slice(ft * T, (ft + 1) * T)
            return slice(ft * T + qt * 512, ft * T + qt * 512 + 512)

        def k_ap(ft, kt):   # [128,128] padded key tile kt (0..11) of feature ft
            return Kp[:, ft * KW + kt * 128: ft * KW + (kt + 1) * 128]

        def v_ap(kt):       # [128, VW] padded V token tile kt
            return Vp[:, kt * VW:(kt + 1) * VW]



        nc.vector.memset(ones_c[:], 1.0)
        nc.vector.memset(ones_r[:], 1.0)
        nc.vector.memset(oh_c[:], 1.0 / H)
        for kt in range(NKT):
            vre = v_ap(kt).rearrange("p (n c) -> p n c", c=VH)
            nc.vector.memset(vre[:, :, HD:HD + 1], 1.0)

        nc.sync.dma_start(out=gv[:], in_=gvec[:])
        nc.sync.dma_start(out=dnb2[:], in_=dn_b2[:])
        for i in range(8):
            nc.sync.dma_start(out=mk[:, i * 128:(i + 1) * 128], in_=mks[i])

        def m_ap(i):        # mask tile i
            return mk[:, i * 128:(i + 1) * 128]

        # mask index for (qi, j) slot; j in 0..4, kt = qi + j. None = no mask.
        def mask_idx(qi, j):
            if qi == 0 and j == 0:
                return 2
            if qi == 0 and j == 1:
                return 3
            if qi == 1 and j == 0:
                return 4
            if qi == 7 and j == 4:
                return 5
            if qi == 7 and j == 3:
                return 6
            if qi == 6 and j == 4:
                return 7
            if j == 0:
                return 0
            if j == 4:
                return 1
            return None

        # ---------------- helpers ----------------
        def w_proj(w_dram, rhs_cols, out_fn, qts=(0, 1), wtiles=None):
            """out_fn(mt, qt, psum) for psum = W.T @ rhs over HT k-tiles."""
            if wtiles is None:
                wtiles = []
                for k in range(HT):
                    t = wp.tile([128, H], dt.bfloat16, tag="w768")
                    nc.sync.dma_start(out=t[:], in_=w_dram[k * 128:(k + 1) * 128, :])
                    wtiles.append(t)
            for qt in qts:
                for mt in range(HT):
                    pt = psW.tile([128, 512], dt.float32, tag="work")
                    for k in range(HT):
                        nc.tensor.matmul(
                            pt[:], wtiles[k][:, mt * 128:(mt + 1) * 128],
                            rhs_cols(k, qt),
                            start=(k == 0), stop=(k == HT - 1))
                    out_fn(mt, qt, pt)
            return wtiles

        # layernorm of one 512-token half qt: updates h, writes hb.
        # hb slices hold the pre-norm bf16 copy during stats (overwritten with
        # the normalized value at the end); ob slices are square scratch.
        # rsqrt is computed on DVE (magic-constant seed + Newton) so the
        # scalar engine never loads the Ln/Sqrt activation tables.
        MAGIC = float(0x5F3759DF)

        def layernorm_half(g_col, b_col, qt):
            # stats inputs: bf16 copy of h + its squares. columns pre-scaled
            # by 1/H so mp/sp are E[x] / E[x^2] directly.
            for ft in range(HT):
                nc.vector.tensor_copy(hb[:, hs(ft, qt)], h[:, hs(ft, qt)])
            for ft in range(HT):
                nc.vector.tensor_mul(ob[:, hs(ft, qt)], hb[:, hs(ft, qt)],
                                     hb[:, hs(ft, qt)])
            mp = psW.tile([1, 512], dt.float32, tag="work", name="mp")
            for ft in range(HT):
                nc.tensor.matmul(mp[:], oh_c[:], hb[:, hs(ft, qt)],
                                 start=(ft == 0), stop=(ft == HT - 1))
            sp = psW.tile([1, 512], dt.float32, tag="work", name="sp")
            for ft in range(HT):
                nc.tensor.matmul(sp[:], oh_c[:], ob[:, hs(ft, qt)],
                                 start=(ft == 0), stop=(ft == HT - 1))
            mrow = rp.tile([1, 512], dt.bfloat16, tag="mrow")
            nc.vector.tensor_copy(mrow[:], mp[:])
            m2 = rp.tile([1, 512], dt.float32, tag="m2")
            nc.vector.tensor_mul(m2[:], mp[:], mrow[:])
            spn = rp.tile([1, 512], dt.float32, tag="spn")
            nc.vector.tensor_scalar_add(spn[:], sp[:], EPS)
            var = rp.tile([1, 512], dt.float32, tag="var")
            nc.vector.scalar_tensor_tensor(var[:], m2[:], -1.0, spn[:],
                                           OP.mult, OP.add)
            rrow = rp.tile([1, 512], dt.bfloat16, tag="rrow")
            # y0 = bitcast(MAGIC - (bits(var) >> 1)); one Newton step.
            sd = rp.tile([1, 512], dt.float32, tag="sd")
            nc.vector.tensor_scalar(sd[:].bitcast(dt.int32),
                                    var[:].bitcast(dt.int32), -0.5,
                                    MAGIC, OP.mult, OP.add)
            y0 = sd[:]
            t1 = rp.tile([1, 512], dt.float32, tag="t1")
            nc.vector.tensor_mul(t1[:], y0, y0)
            nc.vector.tensor_mul(t1[:], t1[:], var[:])
            nc.vector.tensor_scalar(t1[:], t1[:], -0.5, 1.5, OP.mult, OP.add)
            nc.vector.tensor_mul(rrow[:], y0, t1[:])
            mbs = rp.tile([128, 512], dt.bfloat16, tag="mbs")
            nc.gpsimd.partition_broadcast(mbs[:], mrow[:])
            rbs = rp.tile([128, 512], dt.bfloat16, tag="rbs")
            nc.gpsimd.partition_broadcast(rbs[:], rrow[:])
            for ft in range(HT):
                sl = hs(ft, qt)
                t2 = rp.tile([128, 512], dt.float32, tag="t2")
                nc.vector.tensor_sub(t2[:], h[:, sl], mbs[:])
                nc.vector.tensor_mul(t2[:], t2[:], rbs[:])
                nc.scalar.activation(h[:, sl], t2[:], AF.Identity,
                                     scale=g_col(ft), bias=b_col(ft))
                nc.scalar.activation(hb[:, sl], t2[:], AF.Identity,
                                     scale=g_col(ft), bias=b_col(ft))

        # ---------------- input projection ----------------
        for ft in range(HT):
            nc.sync.dma_start(out=h[:, hs(ft)], in_=pe[ft * 128:(ft + 1) * 128, :])
        xb = rp.tile([128, T], dt.bfloat16, tag="xb", bufs=1)
        nc.sync.dma_start(out=xb[:], in_=xT[:])

        w1t = wp.tile([128, H], dt.bfloat16, tag="w768")
        nc.sync.dma_start(out=w1t[:], in_=up_w1[:])
        for mt in range(HT):
            for qt in range(2):
                pt = psW.tile([128, 512], dt.float32, tag="work")
                nc.tensor.matmul(pt[:], w1t[:, mt * 128:(mt + 1) * 128],
                                 xb[:, qt * 512:qt * 512 + 512],
                                 start=True, stop=True)
                nc.scalar.activation(ob[:, hs(mt, qt)], pt[:], AF.Tanh,
                                     bias=gv[:, 0 * HT + mt:0 * HT + mt + 1])

        def up2_out(mt, qt, pt):
            sl = hs(mt, qt)
            nc.vector.scalar_tensor_tensor(
                h[:, sl], pt[:], gv[:, 1 * HT + mt:1 * HT + mt + 1], h[:, sl],
                OP.add, OP.add)
        w_proj(up_w2, lambda k, qt: ob[:, hs(k, qt)], up2_out)

        for qt in range(2):
            layernorm_half(lambda ft: gv[:, 2 * HT + ft:2 * HT + ft + 1],
                           lambda ft: gv[:, 3 * HT + ft:3 * HT + ft + 1], qt)

        # ---------------- encoder layers ----------------
        for l in range(n_layers):
            bv_t = bp.tile([128, 10 * HT], dt.float32, tag="bvec")
            nc.sync.dma_start(out=bv_t[:], in_=vecs[l])
            bi_t = bp.tile([128, FT], dt.float32, tag="bivec")
            nc.sync.dma_start(out=bi_t[:], in_=bi_all[l])

            def vcol(i, ft):
                return bv_t[:, i * HT + ft:i * HT + ft + 1]

            # --- K projection (interior only; evict into padded interior) ---
            def k_out(mt, qt, pt):
                sl = slice(mt * KW + 256 + qt * 512, mt * KW + 256 + qt * 512 + 512)
                nc.vector.tensor_scalar_add(Kp[:, sl], pt[:], vcol(1, mt))
            w_proj(Wk[l], lambda k, qt: hb[:, hs(k, qt)], k_out)

            # --- Q projection ---
            def q_out(mt, qt, pt):
                nc.vector.tensor_scalar_add(qb[:, hs(mt, qt)], pt[:], vcol(0, mt))
            w_proj(Wq[l], lambda k, qt: hb[:, hs(k, qt)], q_out)

            # --- V projection (interior, token-major: h stationary) ---
            wvt = []
            for k in range(HT):
                t = wp.tile([128, H], dt.bfloat16, tag="w768")
                nc.sync.dma_start(out=t[:], in_=Wv[l, k * 128:(k + 1) * 128, :])
                wvt.append(t)
            for vt in range(2, 10):
                tt = vt - 2
                for n0, nn in ((0, 512), (512, 256)):
                    pt = psW.tile([128, 512], dt.float32, tag="work")
                    for k in range(HT):
                        nc.tensor.matmul(
                            pt[:, :nn],
                            hb[:, k * T + tt * 128: k * T + tt * 128 + 128],
                            wvt[k][:, n0:n0 + nn],
                            start=(k == 0), stop=(k == HT - 1))
                    dst = v_ap(vt).rearrange("p (n c) -> p n c", c=VH)
                    h0, nh_ = n0 // HD, nn // HD
                    src = pt[:, :nn].rearrange("p (n c) -> p n c", c=HD)
                    nc.scalar.activation(dst[:, h0:h0 + nh_, 0:HD], src[:], AF.Copy)

            # --- attention, grouped by 512-token halves ---
            att_cm = tc.tile_pool(name="attps", bufs=2, space="PSUM")
            att_ps = att_cm.__enter__()

            # per-qi tile: 5 key tiles of 128, N=128 matmuls; 0/1 masks
            # zero out-of-window p entries after the exp (DVE).
            def attn_tile(qi, hd_i, pv, qoff):
                ft, r0 = hd_i // 2, (hd_i % 2) * HD
                st = att_ps.tile([128, 640], dt.float32, tag="st", bufs=2)
                for j in range(5):
                    kt = qi + j
                    nc.tensor.matmul(
                        st[:, j * 128:(j + 1) * 128],
                        k_ap(ft, kt)[r0:r0 + HD, :],
                        qb[r0:r0 + HD, ft * T + qi * 128: ft * T + qi * 128 + 128],
                        start=True, stop=True)
                p = pbp.tile([128, 640], dt.bfloat16, tag="pb")
                nc.scalar.activation(p[:], st[:], AF.Exp, scale=ISQ)
                for j in range(5):
                    mi = mask_idx(qi, j)
                    if mi is not None:
                        nc.vector.tensor_mul(p[:, j * 128:(j + 1) * 128],
                                             p[:, j * 128:(j + 1) * 128],
                                             m_ap(mi))
                for j in range(5):
                    kt = qi + j
                    nc.tensor.matmul(
                        pv[:, qoff:qoff + 128],
                        v_ap(kt)[:, hd_i * VH:(hd_i + 1) * VH],
                        p[:, j * 128:(j + 1) * 128],
                        start=(j == 0), stop=(j == 4))

            def attn_norm(segs, w, hd_i, pv):
                ft, r0 = hd_i // 2, (hd_i % 2) * HD
                dn = rp.tile([1, 512], dt.float32, tag="dnr")
                nc.scalar.activation(dn[0:1, 0:w], pv[HD:VH, 0:w], AF.Copy)
                r = rp.tile([1, 512], dt.float32, tag="rrec")
                nc.vector.reciprocal_approx_fast(r[0:1, 0:w], dn[0:1, 0:w])
                rb_ = rp.tile([1, 512], dt.bfloat16, tag="rbb")
                nc.vector.tensor_copy(rb_[0:1, 0:w], r[0:1, 0:w])
                bcs = rp.tile([HD, 512], dt.bfloat16, tag="bcs")
                nc.gpsimd.partition_broadcast(bcs[:, 0:w], rb_[0:1, 0:w])
                for pc, oc, sw in segs:
                    nc.vector.tensor_mul(
                        ob[r0:r0 + HD, ft * T + oc: ft * T + oc + sw],
                        pv[0:HD, pc:pc + sw], bcs[:, pc:pc + sw])

            def o_out_g(g):
                def o_out(mt, qt, pt):
                    sl = hs(mt, qt)
                    nc.vector.scalar_tensor_tensor(
                        h[:, sl], pt[:], vcol(3, mt), h[:, sl], OP.add, OP.add)
                return o_out

            # FFN split: ffn1 produces fmid chunks (kept in fp pool),
            # ffn2 consumes them into the fpt accumulators.
            def ffn1_half(g, store):
                for ch in range(6):
                    wi_ch = []
                    for k in range(HT):
                        t = wip.tile([128, 512], dt.bfloat16, tag="wi")
                        nc.sync.dma_start(
                            out=t[:],
                            in_=Wi[l, k * 128:(k + 1) * 128,
                                   ch * 512:(ch + 1) * 512])
                        wi_ch.append(t)
                    fmid = fp.tile([128, 4 * 512], dt.bfloat16, tag="fmid")
                    store.append(fmid)
                    for mi in range(4):
                        mt = ch * 4 + mi
                        pt = psW.tile([128, 512], dt.float32, tag="work")
                        for k in range(HT):
                            nc.tensor.matmul(
                                pt[:], wi_ch[k][:, mi * 128:(mi + 1) * 128],
                                hb[:, hs(k, g)],
                                start=(k == 0), stop=(k == HT - 1))
                        nc.scalar.activation(fmid[:, mi * 512:(mi + 1) * 512],
                                             pt[:], AF.Gelu,
                                             bias=bi_t[:, mt:mt + 1])

            def ffn2_half(g, store, ffn_ps):
                fpt = [ffn_ps.tile([128, 512], dt.float32, tag=f"ff2_{m}",
                                   name=f"ff2_{m}_{g}") for m in range(HT)]
                for ch in range(6):
                    fmid = store[ch]
                    wo2_ch = []
                    for mi in range(4):
                        t = wp.tile([128, H], dt.bfloat16, tag="w768")
                        nc.sync.dma_start(
                            out=t[:],
                            in_=Wo2[l, (ch * 4 + mi) * 128:(ch * 4 + mi + 1) * 128, :])
                        wo2_ch.append(t)
                    for m in range(HT):
                        for mi in range(4):
                            kt = ch * 4 + mi
                            nc.tensor.matmul(
                                fpt[m][:], wo2_ch[mi][:, m * 128:(m + 1) * 128],
                                fmid[:, mi * 512:(mi + 1) * 512],
                                start=(kt == 0), stop=(kt == FT - 1))
                for m in range(HT):
                    sl = hs(m, g)
                    nc.vector.scalar_tensor_tensor(h[:, sl], fpt[m][:], vcol(6, m),
                                                   h[:, sl], OP.add, OP.add)

            # ---- emission schedule ----
            # attention g0 (interior qtiles first), then O/LN1/FFN1 g0
            # overlapping attention g1.
            fm0, fm1 = [], []
            # interior quad (qi 2..5, tokens 256:768): no halo dependency ->
            # overlaps the AllGather emitted after it.
            for hd_i in range(NH):
                pv = att_ps.tile([VH, 512], dt.float32, tag="pv",
                                 name=f"pvI_{hd_i}", bufs=2)
                for qi in (2, 3, 4, 5):
                    attn_tile(qi, hd_i, pv, (qi - 2) * 128)
                attn_norm([(0, 256, 512)], 512, hd_i, pv)
            # --- K/V edge exchange (K edges: tok 0:256 and 768:1024) ---
            for ft in range(HT):
                nc.sync.dma_start(
                    out=cc_send[KL + ft * 128 * 256: KL + (ft + 1) * 128 * 256]
                    .rearrange("(p t) -> p t", p=128),
                    in_=Kp[:, ft * KW + 256: ft * KW + 512])
                nc.sync.dma_start(
                    out=cc_send[KR + ft * 128 * 256: KR + (ft + 1) * 128 * 256]
                    .rearrange("(p t) -> p t", p=128),
                    in_=Kp[:, ft * KW + 1024: ft * KW + 1280])
            nc.sync.dma_start(
                out=cc_send[VL:VL + VSEG].rearrange("(p t) -> p t", p=128),
                in_=Vp[:, 2 * VW:4 * VW])
            nc.sync.dma_start(
                out=cc_send[VR:VR + VSEG].rearrange("(p t) -> p t", p=128),
                in_=Vp[:, 8 * VW:10 * VW])
            nc.gpsimd.collective_compute(
                "AllGather", OP.bypass, ins=[cc_send[:]], outs=[cc_recv[:]],
                replica_groups=[[0, 1], [2, 3], [4, 5], [6, 7]])
            # left halo <- rank0's right edge ; right halo <- rank1's left edge
            for ft in range(HT):
                nc.sync.dma_start(
                    out=Kp[:, ft * KW: ft * KW + 256],
                    in_=cc_recv[0, KR + ft * 128 * 256: KR + (ft + 1) * 128 * 256]
                    .rearrange("(p t) -> p t", p=128))
                nc.sync.dma_start(
                    out=Kp[:, ft * KW + 1280: ft * KW + 1536],
                    in_=cc_recv[1, KL + ft * 128 * 256: KL + (ft + 1) * 128 * 256]
                    .rearrange("(p t) -> p t", p=128))
            nc.sync.dma_start(
                out=Vp[:, 0:2 * VW],
                in_=cc_recv[0, VR:VR + VSEG].rearrange("(p t) -> p t", p=128))
            nc.sync.dma_start(
                out=Vp[:, 10 * VW:12 * VW],
                in_=cc_recv[1, VL:VL + VSEG].rearrange("(p t) -> p t", p=128))

            # edge quad (qi 0,1 left halo / 6,7 right halo) after the CC.
            for hd_i in range(NH):
                pv = att_ps.tile([VH, 512], dt.float32, tag="pv",
                                 name=f"pvE_{hd_i}", bufs=2)
                for qi in (6, 7, 1, 0):
                    attn_tile(qi, hd_i, pv,
                              qi * 128 if qi < 2 else (qi - 4) * 128)
                attn_norm([(0, 0, 256), (256, 768, 256)], 512, hd_i, pv)
            wo_t = w_proj(Wo[l], lambda k, qt: ob[:, hs(k, qt)],
                          o_out_g(0), qts=(0,))
            layernorm_half(lambda ft: vcol(4, ft), lambda ft: vcol(5, ft), 0)
            # FFN1(g0) emitted below attention-g1 in priority: its dense
            # matmuls fill the exp-latency bubbles of attention g1.
            ffn1_half(0, fm0)
            w_proj(Wo[l], lambda k, qt: ob[:, hs(k, qt)], o_out_g(1), qts=(1,),
                   wtiles=wo_t)
            layernorm_half(lambda ft: vcol(4, ft), lambda ft: vcol(5, ft), 1)
            att_cm.__exit__(None, None, None)

            ffn_cm = tc.tile_pool(name="ffps", bufs=1, space="PSUM")
            ffn_ps = ffn_cm.__enter__()
            ffn2_half(0, fm0, ffn_ps)
            ffn1_half(1, fm1)
            layernorm_half(lambda ft: vcol(7, ft), lambda ft: vcol(8, ft), 0)
            ffn2_half(1, fm1, ffn_ps)
            ffn_cm.__exit__(None, None, None)
            layernorm_half(lambda ft: vcol(7, ft), lambda ft: vcol(8, ft), 1)

        # ---------------- output projection ----------------
        def d1_out(mt, qt, pt):
            nc.scalar.activation(ob[:, hs(mt, qt)], pt[:], AF.Tanh,
                                 bias=gv[:, 4 * HT + mt:4 * HT + mt + 1])
        w_proj(dn_w1, lambda k, qt: hb[:, hs(k, qt)], d1_out)

        w2t = wp.tile([128, HT * C], dt.bfloat16, tag="w768")
        for k in range(HT):
            nc.sync.dma_start(out=w2t[:, k * C:(k + 1) * C],
                              in_=dn_w2[k * 128:(k + 1) * 128, :])
        for qt in range(2):
            pt = psW.tile([128, 512], dt.float32, tag="work")
            for k in range(HT):
                nc.tensor.matmul(pt[:], w2t[:, k * C:(k + 1) * C],
                                 ob[:, hs(k, qt)], start=(k == 0),
                                 stop=(k == HT - 1))
            yo = rp.tile([128, 512], dt.float32, tag="yout")
            nc.scalar.activation(yo[:], pt[:], AF.Identity, bias=dnb2[:])
            nc.sync.dma_start(out=y[:, qt * 512:qt * 512 + 512], in_=yo[:])

    nc.compile()
    return nc


def _host_prep(inputs, n_layers):
    f32 = np.float32
    x = np.asarray(inputs["x"], f32)
    ts = np.asarray(inputs["timesteps"])
    half = C // 2
    freqs = np.exp(-np.log(10000.0) * np.arange(half, dtype=f32) / half)
    a = ts.astype(f32)[:, None] * freqs[None, :]
    emb0 = np.concatenate([np.cos(a), np.sin(a)], -1).astype(f32)
    t1 = emb0 @ np.asarray(inputs["t_w1"], f32) + np.asarray(inputs["t_b1"], f32)
    t1 = t1 / (1.0 + np.exp(-t1))
    emb = (t1 @ np.asarray(inputs["t_w2"], f32) + np.asarray(inputs["t_b2"], f32)).astype(f32)

    def cvt(w):
        return np.ascontiguousarray(np.asarray(w, f32).astype(bf16))

    def packvec(v, nt):
        return np.ascontiguousarray(np.asarray(v, f32).reshape(nt, 128).T)

    com = dict(
        up_w1=cvt(inputs["up_w1"]), up_w2=cvt(inputs["up_w2"]),
        dn_w1=cvt(inputs["down_w1"]), dn_w2=cvt(inputs["down_w2"]),
        Wq=cvt(inputs["Wq"][:n_layers]), Wk=cvt(inputs["Wk"][:n_layers]),
        Wv=cvt(inputs["Wv"][:n_layers]), Wo=cvt(inputs["Wo"][:n_layers]),
        Wi=cvt(inputs["Wi"][:n_layers]), Wo2=cvt(inputs["Wo2"][:n_layers]),
        dn_b2=np.ascontiguousarray(np.asarray(inputs["down_b2"], f32).reshape(1, C).T),
    )
    # bv folded into bo: boE = bo + bv @ Wo
    boE = np.stack([
        np.asarray(inputs["bo"], f32)[ll]
        + np.asarray(inputs["bv"], f32)[ll] @ np.asarray(inputs["Wo"], f32)[ll]
        for ll in range(n_layers)])
    zero = np.zeros((n_layers, H), f32)
    vec_srcs = [np.asarray(inputs["bq"], f32)[:n_layers],
                np.asarray(inputs["bk"], f32)[:n_layers], zero, boE,
                np.asarray(inputs["g1"], f32)[:n_layers],
                np.asarray(inputs["b1"], f32)[:n_layers],
                np.asarray(inputs["bo2"], f32)[:n_layers],
                np.asarray(inputs["g2"], f32)[:n_layers],
                np.asarray(inputs["b2"], f32)[:n_layers], zero]
    vecs = np.stack([
        np.concatenate([packvec(src[ll], HT) for src in vec_srcs], axis=1)
        for ll in range(n_layers)])
    com["vecs"] = np.ascontiguousarray(vecs.astype(f32))
    com["bi_all"] = np.ascontiguousarray(
        np.stack([packvec(np.asarray(inputs["bi"], f32)[ll], FT)
                  for ll in range(n_layers)]).astype(f32))
    com["gvec"] = np.ascontiguousarray(np.concatenate([
        packvec(inputs["up_b1"], HT), packvec(inputs["up_b2"], HT),
        packvec(inputs["ln_g"], HT), packvec(inputs["ln_b"], HT),
        packvec(inputs["down_b1"], HT), packvec(inputs["down_b1"], HT)],
        axis=1).astype(f32))

    pos = np.asarray(inputs["pos_emb"], f32)

    # multiplicative 0/1 mask tiles [8,128,128]: p = key-in-tile, f = query
    pi = np.arange(128)[:, None]
    fi = np.arange(128)[None, :]
    ML = (pi >= fi).astype(f32)
    MR = (pi <= fi).astype(f32)
    ZZ = np.ones((128, 128), f32)
    NN = np.zeros((128, 128), f32)
    mk_sh = {}
    # order: ML MR q0kt0 q0kt1 q1kt1 q7kt11 q7kt10 q6kt10
    mk_sh[0] = np.stack([ML, MR, NN, NN, NN, MR, ZZ, MR])
    mk_sh[1] = np.stack([ML, MR, ML, ZZ, ML, NN, NN, NN])
    for sh in mk_sh:
        mk_sh[sh] = np.ascontiguousarray(mk_sh[sh].astype(bf16))

    in_maps = []
    for c in range(8):
        b, sh = c // 2, c % 2
        sl = slice(sh * T, (sh + 1) * T)
        im = dict(com)
        im["xT"] = np.ascontiguousarray(x[b, sl].T.astype(bf16))
        im["pe"] = np.ascontiguousarray((pos[sl] + emb[b][None, :]).T.astype(f32))
        im["mks"] = mk_sh[sh]
        in_maps.append(im)
    return in_maps


def kernel(**inputs):
    from concourse.bass_utils import run_bass_kernel_spmd

    n_layers = L
    if n_layers not in _CACHED:
        _CACHED[n_layers] = _build(n_layers)
    nc = _CACHED[n_layers]
    in_maps = _host_prep(inputs, n_layers)
    trace = os.environ.get("KERNEL_TRACE", "0") == "1"
    tmpdir = os.environ.get("KERNEL_TMPDIR") or None
    res = run_bass_kernel_spmd(nc, in_maps, list(range(8)), trace=trace,
                               tmpdir=tmpdir)
    global LAST_EXEC_NS
    if getattr(res, "exec_time_ns", None):
        LAST_EXEC_NS = res.exec_time_ns
    out = np.empty((B, S, C), np.float32)
    for c in range(8):
        b, sh = c // 2, c % 2
        out[b, sh * T:(sh + 1) * T, :] = res.results[c]["y"].T
    return out

